# revision 11
# baseline (speedup 1.0000x reference)
"""ContentAttention kernel for 8 Trainium2 NeuronCores.

Computation (per batch b):
    h_att  = h[b] @ W_h2att + b_h2att                  # [512]
    e      = tanh(p_att_feats[b] + h_att)              # [1024, 512]
    scores = e @ w_alpha (+ b_alpha, dropped: softmax shift-invariant)
    w      = softmax(scores)                           # [1024]
    out[b] = w @ att_feats[b]                          # [1024]

Sharding: data-parallel over batch B=128 -> 16 batches/core x 8 cores.
Params are tiny and replicated.

The kernel is HBM-DMA-bound, so the two big streams (p_att_feats,
att_feats) are cast to bf16 on the host before upload: 48MB of HBM
reads per core instead of 96MB (output rel-err ~2.6e-3, well under the
2e-2 gate; accumulations stay fp32 in PSUM / DVE accum registers).

Per-core design:
  - regions are mapped partition-major (r = p*8 + j) so each batch's
    slab is one contiguous 8KB (p) / 16KB (att) run per partition ->
    one DMA descriptor per partition per slab. Softmax and the
    weighted sum are invariant to any consistent region permutation.
  - p slab [128, 8, 512] bf16 (1MB, one DMA on the ACT HWDGE ring);
    att slab [128, 8, 1024] bf16 (2MB, one DMA on the SP HWDGE ring).
  - h_att[b] broadcast across partitions via a PE ones-matmul into
    PSUM + ACT copy to bf16 SBUF (no HBM-amplified stride-0 DMA).
  - DVE add -> ACT tanh -> per-chunk DVE scalar_tensor_tensor
    (fused *w_alpha multiply + free-dim sum) giving [128,1] score cols.
  - softmax over 1024 scores held as [128, 8]: cross-partition max/sum
    via gpsimd partition_all_reduce; weights downcast to bf16.
  - weighted sum over regions on PE: lhsT = weight column [128,1] bf16,
    rhs = att slab chunk [128, 512] bf16, accumulated over 8 region
    chunks into PSUM [1, 512] x2.
"""

import numpy as np

B, R, K_H, D, F = 128, 1024, 1024, 512, 1024
N_CORES = 8
BPC = B // N_CORES  # batches per core
RC = R // 128  # region chunks per batch (r = p*RC + j)
KC = K_H // 128

_cached = {}


def _build_program():
    from contextlib import ExitStack

    import concourse.bass as bass
    import concourse.bass_isa as bass_isa
    import concourse.tile as tile
    from concourse import bacc, mybir

    f32 = mybir.dt.float32
    bf16 = mybir.dt.bfloat16
    AF = mybir.ActivationFunctionType
    ALU = mybir.AluOpType
    AX = mybir.AxisListType

    def bcast_p(row_ap, parts):
        # replicate a [1, n] DRAM row across `parts` partitions
        return bass.AP(
            tensor=row_ap.tensor,
            offset=row_ap.offset,
            ap=[[0, parts], list(row_ap.ap[-1])],
        )

    def bcast_mid(t, c):
        # [128, D] sbuf tile -> [128, c, D] view with 0-stride middle dim
        return bass.AP(
            tensor=t.tensor,
            offset=t.offset,
            ap=[list(t.ap[0]), [0, c], list(t.ap[1])],
        )

    nc = bacc.Bacc("TRN2", target_bir_lowering=False, debug=False)
    oh_ap = nc.dram_tensor("onehots", [BPC, BPC * 128], f32, kind="ExternalInput").ap()
    h_ap = nc.dram_tensor("h", [BPC, K_H], f32, kind="ExternalInput").ap()
    att_ap = nc.dram_tensor("att", [BPC, R, F], bf16, kind="ExternalInput").ap()
    p_ap = nc.dram_tensor("p", [BPC, R, D], bf16, kind="ExternalInput").ap()
    w_ap = nc.dram_tensor("w_h2att", [K_H, D], f32, kind="ExternalInput").ap()
    b2_ap = nc.dram_tensor("b_h2att", [1, D], f32, kind="ExternalInput").ap()
    wa_ap = nc.dram_tensor("w_alpha", [1, D], bf16, kind="ExternalInput").ap()
    out_ap = nc.dram_tensor("out", [BPC, F], f32, kind="ExternalOutput").ap()

    with tile.TileContext(nc) as tc, ExitStack() as ctx:
        consts = ctx.enter_context(tc.tile_pool(name="consts", bufs=1))
        wpool = ctx.enter_context(tc.tile_pool(name="wpool", bufs=1))
        ppool = ctx.enter_context(tc.tile_pool(name="ppool", bufs=5))
        apool = ctx.enter_context(tc.tile_pool(name="apool", bufs=5))
        hbpool = ctx.enter_context(tc.tile_pool(name="hbpool", bufs=3))
        spool = ctx.enter_context(tc.tile_pool(name="spool", bufs=6))
        outp = ctx.enter_context(tc.tile_pool(name="outp", bufs=4))
        ps_mm = ctx.enter_context(tc.tile_pool(name="ps_mm", bufs=2, space="PSUM"))
        ps_bc = ctx.enter_context(tc.tile_pool(name="ps_bc", bufs=2, space="PSUM"))
        ps_mic = ctx.enter_context(tc.tile_pool(name="ps_mic", bufs=1, space="PSUM"))

        walpha_bc = consts.tile([128, D], bf16)
        nc.sync.dma_start(out=walpha_bc, in_=bcast_p(wa_ap, 128))

        # ---- phase 0: h_att = h @ W + b_h2att, kept in SBUF
        w_all = wpool.tile([128, KC, D], f32, tag="w")
        nc.sync.dma_start(
            out=w_all, in_=w_ap.rearrange("(kc p) d -> p kc d", p=128)
        )
        h_nat = wpool.tile([BPC, K_H], f32, tag="hnat")
        nc.scalar.dma_start(out=h_nat, in_=h_ap)
        ident16 = consts.tile([BPC, BPC], f32)
        from concourse.masks import make_identity

        make_identity(nc, ident16)
        # onehots[:, b*128:(b+1)*128] is delta_{k,b} as a [16, 128] lhsT:
        # matmul against hatt broadcasts row b across all 128 partitions.
        onehots = consts.tile([BPC, BPC * 128], f32)
        nc.gpsimd.dma_start(out=onehots, in_=oh_ap)
        # transpose h on PE: [16, 128] chunks -> [128, 16]
        hT_all = wpool.tile([128, KC, BPC], f32, tag="hT")
        for kc in range(KC):
            tr_ps = ps_mm.tile([128, BPC], f32, tag="ps0")
            nc.tensor.transpose(
                tr_ps, h_nat[:, kc * 128 : (kc + 1) * 128], ident16
            )
            nc.scalar.copy(hT_all[:, kc, :], tr_ps)
        ps_hatt = ps_mic.tile([BPC, D], f32, tag="mic")
        for kc in range(KC):
            nc.tensor.matmul(
                ps_hatt,
                lhsT=hT_all[:, kc, :],
                rhs=w_all[:, kc, :],
                start=(kc == 0),
                stop=(kc == KC - 1),
            )
        b2_bc = consts.tile([BPC, D], f32)
        nc.gpsimd.dma_start(out=b2_bc, in_=bcast_p(b2_ap, BPC))
        hatt = consts.tile([BPC, D], f32)
        nc.vector.tensor_add(hatt, ps_hatt, b2_bc)

        # ---- main loop over this core's batches
        # Both big DMA streams are issued by the sync (SP) engine, which
        # runs no compute: issue order is p0,a0,p1,a1,... gated only by
        # pool-slot semaphores, so the loads run several batches ahead of
        # compute instead of being serialized behind same-engine ops.
        for b in range(BPC):
            p_t = ppool.tile([128, RC, D], bf16, tag="p")
            nc.sync.dma_start(
                out=p_t, in_=p_ap[b].rearrange("(p j) d -> p j d", p=128)
            )
            a_t = apool.tile([128, RC, F], bf16, tag="a")
            nc.sync.dma_start(
                out=a_t, in_=att_ap[b].rearrange("(p j) f -> p j f", p=128)
            )

            # broadcast h_att[b] to 128 partitions on PE, downcast to bf16
            bc_ps = ps_bc.tile([128, D], f32, tag="bc")
            nc.tensor.matmul(
                bc_ps,
                lhsT=onehots[:, b * 128 : (b + 1) * 128],
                rhs=hatt,
                start=True,
                stop=True,
            )
            hb = hbpool.tile([128, D], bf16)
            nc.scalar.copy(hb, bc_ps)

            nc.vector.tensor_add(p_t, p_t, bcast_mid(hb, RC))
            nc.scalar.activation(p_t, p_t, AF.Tanh)
            scores = spool.tile([128, RC], f32, tag="scores")
            for j in range(RC):
                # out = (e * 1.0) * w_alpha; accum_out = sum -> score col
                nc.vector.scalar_tensor_tensor(
                    out=p_t[:, j, :],
                    in0=p_t[:, j, :],
                    scalar=1.0,
                    in1=walpha_bc,
                    op0=ALU.mult,
                    op1=ALU.mult,
                    accum_out=scores[:, j : j + 1],
                )

            # softmax over the 1024 scores laid out as [128 partitions, RC].
            # No max-subtraction: |score| <= sum|w_alpha| (~18, tanh in
            # [-1,1]) so exp cannot overflow fp32. exp+row-sum fused on ACT.
            expb = spool.tile([128, RC], f32, tag="expb")
            s1 = spool.tile([128, 1], f32, tag="s1")
            nc.scalar.activation(expb, scores, AF.Exp, accum_out=s1)
            sm = spool.tile([128, 1], f32, tag="sm")
            nc.gpsimd.partition_all_reduce(
                sm, s1, channels=128, reduce_op=bass_isa.ReduceOp.add
            )
            rec = spool.tile([128, 1], f32, tag="rec")
            nc.vector.reciprocal(rec, sm)
            wgt = spool.tile([128, RC], bf16, tag="wgt")
            nc.vector.tensor_scalar_mul(wgt, expb, rec)

            # ---- phase 2: out[b] = weight @ att_feats[b]
            ps0 = ps_mm.tile([1, 512], f32, tag="ps0")
            ps1 = ps_mm.tile([1, 512], f32, tag="ps1")
            for j in range(RC):
                nc.tensor.matmul(
                    ps0,
                    lhsT=wgt[:, j : j + 1],
                    rhs=a_t[:, j, 0:512],
                    start=(j == 0),
                    stop=(j == RC - 1),
                )
                nc.tensor.matmul(
                    ps1,
                    lhsT=wgt[:, j : j + 1],
                    rhs=a_t[:, j, 512:1024],
                    start=(j == 0),
                    stop=(j == RC - 1),
                )
            ob = outp.tile([1, F], f32)
            nc.scalar.copy(ob[:, 0:512], ps0)
            nc.scalar.copy(ob[:, 512:1024], ps1)
            nc.gpsimd.dma_start(out=out_ap[b : b + 1, :], in_=ob)

    nc.compile()
    return nc


def _get_program():
    if "nc" not in _cached:
        _cached["nc"] = _build_program()
    return _cached["nc"]


def _make_in_maps(inputs):
    import ml_dtypes

    bf = ml_dtypes.bfloat16
    h = np.ascontiguousarray(np.asarray(inputs["h"], dtype=np.float32))
    att = np.ascontiguousarray(np.asarray(inputs["att_feats"])).astype(bf)
    p = np.ascontiguousarray(np.asarray(inputs["p_att_feats"])).astype(bf)
    W = np.ascontiguousarray(np.asarray(inputs["W_h2att"], dtype=np.float32))
    b2 = np.ascontiguousarray(
        np.asarray(inputs["b_h2att"], dtype=np.float32).reshape(1, D)
    )
    wa = np.asarray(inputs["w_alpha"]).reshape(1, D).astype(bf)
    # b_alpha is a scalar added to every score; softmax is shift-invariant.
    onehots = np.ascontiguousarray(
        np.kron(np.eye(BPC, dtype=np.float32), np.ones((1, 128), dtype=np.float32))
    )
    in_maps = []
    for c in range(N_CORES):
        lo, hi = c * BPC, (c + 1) * BPC
        in_maps.append(
            {
                "onehots": onehots,
                "h": h[lo:hi],
                "att": att[lo:hi],
                "p": p[lo:hi],
                "w_h2att": W,
                "b_h2att": b2,
                "w_alpha": wa,
            }
        )
    return in_maps


def kernel(**inputs) -> np.ndarray:
    from concourse.bass_utils import run_bass_kernel_spmd

    nc = _get_program()
    in_maps = _make_in_maps(inputs)
    res = run_bass_kernel_spmd(nc, in_maps, list(range(N_CORES)))
    out = np.concatenate([res.results[c]["out"] for c in range(N_CORES)], axis=0)
    return out.astype(np.float32)


# revision 13
# speedup vs baseline: 1.0273x; 1.0273x over previous
"""ContentAttention kernel for 8 Trainium2 NeuronCores.

Computation (per batch b):
    h_att  = h[b] @ W_h2att + b_h2att                  # [512]
    e      = tanh(p_att_feats[b] + h_att)              # [1024, 512]
    scores = e @ w_alpha (+ b_alpha, dropped: softmax shift-invariant)
    w      = softmax(scores)                           # [1024]
    out[b] = w @ att_feats[b]                          # [1024]

Sharding: data-parallel over batch B=128 -> 16 batches/core x 8 cores.
Params are tiny and replicated.

The kernel is HBM-DMA-bound, so the two big streams (p_att_feats,
att_feats) and W are cast to bf16 on the host before upload: ~49MB of
HBM reads per core instead of 98MB (output rel-err ~2.8e-3, well under
the 2e-2 gate; accumulations stay fp32 in PSUM / DVE accum registers).

Per-core design:
  - regions are mapped partition-major (r = p*8 + j) so each batch's
    slab is one contiguous 8KB (p) / 16KB (att) run per partition ->
    one DMA descriptor per partition per slab. Softmax and the
    weighted sum are invariant to any consistent region permutation.
  - software-pipelined DMA issue: p slabs (1MB, ACT HWDGE ring) are
    issued 4 batches ahead, att slabs (2MB, SP HWDGE ring) 3 batches
    ahead, so slot-semaphore waits always reference long-consumed
    buffers and the loads never serialize behind same-engine compute.
  - h_att[b] broadcast across partitions via a onehot-selector matmul
    on PE into PSUM + ACT copy to bf16 SBUF.
  - DVE add -> ACT tanh -> per-chunk DVE tensor_tensor_reduce
    (fused *w_alpha multiply + free-dim sum) giving [128,1] score cols.
  - softmax over 1024 scores held as [128, 8], with no max-subtraction:
    |score| <= sum|w_alpha| (~18 since tanh is in [-1,1]), so exp
    cannot overflow fp32. exp + row-sum fused on ACT (accum_out);
    cross-partition sum via gpsimd partition_all_reduce; weights
    downcast to bf16.
  - weighted sum over regions on PE: lhsT = weight column [128,1] bf16,
    rhs = att slab chunk [128, 512] bf16, accumulated over 8 region
    chunks into PSUM [1, 512] x2.
"""

import numpy as np

B, R, K_H, D, F = 128, 1024, 1024, 512, 1024
N_CORES = 8
BPC = B // N_CORES  # batches per core
RC = R // 128  # region chunks per batch (r = p*RC + j)
KC = K_H // 128
PREF_P = 4  # p-slab DMA issue lead (batches)
PREF_A = 3  # att-slab DMA issue lead (batches)

_cached = {}


def _build_program():
    from contextlib import ExitStack

    import concourse.bass as bass
    import concourse.bass_isa as bass_isa
    import concourse.tile as tile
    from concourse import bacc, mybir

    f32 = mybir.dt.float32
    bf16 = mybir.dt.bfloat16
    AF = mybir.ActivationFunctionType
    ALU = mybir.AluOpType

    def bcast_p(row_ap, parts):
        # replicate a [1, n] DRAM row across `parts` partitions
        return bass.AP(
            tensor=row_ap.tensor,
            offset=row_ap.offset,
            ap=[[0, parts], list(row_ap.ap[-1])],
        )

    def bcast_mid(t, c):
        # [128, D] sbuf tile -> [128, c, D] view with 0-stride middle dim
        return bass.AP(
            tensor=t.tensor,
            offset=t.offset,
            ap=[list(t.ap[0]), [0, c], list(t.ap[1])],
        )

    nc = bacc.Bacc("TRN2", target_bir_lowering=False, debug=False)
    oh_ap = nc.dram_tensor("onehots", [BPC, BPC * 128], f32, kind="ExternalInput").ap()
    h_ap = nc.dram_tensor("h", [BPC, K_H], f32, kind="ExternalInput").ap()
    att_ap = nc.dram_tensor("att", [BPC, R, F], bf16, kind="ExternalInput").ap()
    p_ap = nc.dram_tensor("p", [BPC, R, D], bf16, kind="ExternalInput").ap()
    w_ap = nc.dram_tensor("w_h2att", [K_H, D], bf16, kind="ExternalInput").ap()
    b2_ap = nc.dram_tensor("b_h2att", [1, D], f32, kind="ExternalInput").ap()
    wa_ap = nc.dram_tensor("w_alpha", [1, D], bf16, kind="ExternalInput").ap()
    out_ap = nc.dram_tensor("out", [BPC, F], f32, kind="ExternalOutput").ap()

    with tile.TileContext(nc) as tc, ExitStack() as ctx:
        consts = ctx.enter_context(tc.tile_pool(name="consts", bufs=1))
        wpool = ctx.enter_context(tc.tile_pool(name="wpool", bufs=1))
        ppool = ctx.enter_context(tc.tile_pool(name="ppool", bufs=6))
        apool = ctx.enter_context(tc.tile_pool(name="apool", bufs=6))
        hbpool = ctx.enter_context(tc.tile_pool(name="hbpool", bufs=3))
        spool = ctx.enter_context(tc.tile_pool(name="spool", bufs=6))
        outp = ctx.enter_context(tc.tile_pool(name="outp", bufs=3))
        ps_mm = ctx.enter_context(tc.tile_pool(name="ps_mm", bufs=2, space="PSUM"))
        ps_bc = ctx.enter_context(tc.tile_pool(name="ps_bc", bufs=2, space="PSUM"))
        ps_mic = ctx.enter_context(tc.tile_pool(name="ps_mic", bufs=1, space="PSUM"))

        # ---- prefetched big-stream DMAs, issued before any compute
        p_tiles, a_tiles = {}, {}

        def issue_p(b):
            t = ppool.tile([128, RC, D], bf16, tag="p")
            nc.scalar.dma_start(
                out=t, in_=p_ap[b].rearrange("(p j) d -> p j d", p=128)
            )
            p_tiles[b] = t

        def issue_a(b):
            t = apool.tile([128, RC, F], bf16, tag="a")
            nc.sync.dma_start(
                out=t, in_=att_ap[b].rearrange("(p j) f -> p j f", p=128)
            )
            a_tiles[b] = t

        issue_p(0)
        issue_a(0)
        h_nat = wpool.tile([BPC, K_H], f32, tag="hnat")
        nc.scalar.dma_start(out=h_nat, in_=h_ap)
        w_all = wpool.tile([128, KC, D], bf16, tag="w")
        nc.sync.dma_start(
            out=w_all, in_=w_ap.rearrange("(kc p) d -> p kc d", p=128)
        )
        for b in range(1, PREF_P):
            issue_p(b)
        for b in range(1, PREF_A):
            issue_a(b)

        walpha_bc = consts.tile([128, D], bf16)
        nc.gpsimd.dma_start(out=walpha_bc, in_=bcast_p(wa_ap, 128))
        b2_bc = consts.tile([BPC, D], f32)
        nc.gpsimd.dma_start(out=b2_bc, in_=bcast_p(b2_ap, BPC))
        # onehots[:, b*128:(b+1)*128] is delta_{k,b} as a [16, 128] lhsT:
        # matmul against hatt broadcasts row b across all 128 partitions.
        onehots = consts.tile([BPC, BPC * 128], f32)
        nc.gpsimd.dma_start(out=onehots, in_=oh_ap)

        # ---- phase 0: h_att = h @ W + b_h2att, kept in SBUF
        ident16 = consts.tile([BPC, BPC], f32)
        from concourse.masks import make_identity

        make_identity(nc, ident16)
        # transpose h on PE: [16, 128] chunks -> [128, 16], downcast to bf16
        hT_all = wpool.tile([128, KC, BPC], bf16, tag="hT")
        for kc in range(KC):
            tr_ps = ps_mm.tile([128, BPC], f32, tag="ps0")
            nc.tensor.transpose(
                tr_ps, h_nat[:, kc * 128 : (kc + 1) * 128], ident16
            )
            nc.scalar.copy(hT_all[:, kc, :], tr_ps)
        ps_hatt = ps_mic.tile([BPC, D], f32, tag="mic")
        for kc in range(KC):
            nc.tensor.matmul(
                ps_hatt,
                lhsT=hT_all[:, kc, :],
                rhs=w_all[:, kc, :],
                start=(kc == 0),
                stop=(kc == KC - 1),
            )
        hatt = consts.tile([BPC, D], f32)
        nc.vector.tensor_add(hatt, ps_hatt, b2_bc)

        # ---- main loop over this core's batches
        for b in range(BPC):
            if b + PREF_P < BPC:
                issue_p(b + PREF_P)
            if b + PREF_A < BPC:
                issue_a(b + PREF_A)
            p_t = p_tiles.pop(b)
            a_t = a_tiles.pop(b)

            # broadcast h_att[b] to 128 partitions on PE, downcast to bf16
            bc_ps = ps_bc.tile([128, D], f32, tag="bc")
            nc.tensor.matmul(
                bc_ps,
                lhsT=onehots[:, b * 128 : (b + 1) * 128],
                rhs=hatt,
                start=True,
                stop=True,
            )
            hb = hbpool.tile([128, D], bf16)
            nc.scalar.copy(hb, bc_ps)

            nc.vector.tensor_add(p_t, p_t, bcast_mid(hb, RC))
            nc.scalar.activation(p_t, p_t, AF.Tanh)
            scores = spool.tile([128, RC], f32, tag="scores")
            for j in range(RC):
                # out = (e * 1.0) * w_alpha; accum_out = sum -> score col
                nc.vector.scalar_tensor_tensor(
                    out=p_t[:, j, :],
                    in0=p_t[:, j, :],
                    scalar=1.0,
                    in1=walpha_bc,
                    op0=ALU.mult,
                    op1=ALU.mult,
                    accum_out=scores[:, j : j + 1],
                )

            # softmax over the 1024 scores laid out as [128 partitions, RC]
            expb = spool.tile([128, RC], f32, tag="expb")
            s1 = spool.tile([128, 1], f32, tag="s1")
            nc.scalar.activation(expb, scores, AF.Exp, accum_out=s1)
            sm = spool.tile([128, 1], f32, tag="sm")
            nc.gpsimd.partition_all_reduce(
                sm, s1, channels=128, reduce_op=bass_isa.ReduceOp.add
            )
            rec = spool.tile([128, 1], f32, tag="rec")
            nc.vector.reciprocal(rec, sm)
            wgt = spool.tile([128, RC], bf16, tag="wgt")
            nc.vector.tensor_scalar_mul(wgt, expb, rec)

            # ---- phase 2: out[b] = weight @ att_feats[b]
            ps0 = ps_mm.tile([1, 512], f32, tag="ps0")
            ps1 = ps_mm.tile([1, 512], f32, tag="ps1")
            for j in range(RC):
                nc.tensor.matmul(
                    ps0,
                    lhsT=wgt[:, j : j + 1],
                    rhs=a_t[:, j, 0:512],
                    start=(j == 0),
                    stop=(j == RC - 1),
                )
                nc.tensor.matmul(
                    ps1,
                    lhsT=wgt[:, j : j + 1],
                    rhs=a_t[:, j, 512:1024],
                    start=(j == 0),
                    stop=(j == RC - 1),
                )
            ob = outp.tile([1, F], f32)
            nc.scalar.copy(ob[:, 0:512], ps0)
            nc.scalar.copy(ob[:, 512:1024], ps1)
            nc.gpsimd.dma_start(out=out_ap[b : b + 1, :], in_=ob)

    nc.compile()
    return nc


def _get_program():
    if "nc" not in _cached:
        _cached["nc"] = _build_program()
    return _cached["nc"]


def _make_in_maps(inputs):
    import ml_dtypes

    bf = ml_dtypes.bfloat16
    h = np.ascontiguousarray(np.asarray(inputs["h"], dtype=np.float32))
    att = np.ascontiguousarray(np.asarray(inputs["att_feats"])).astype(bf)
    p = np.ascontiguousarray(np.asarray(inputs["p_att_feats"])).astype(bf)
    W = np.ascontiguousarray(np.asarray(inputs["W_h2att"])).astype(bf)
    b2 = np.ascontiguousarray(
        np.asarray(inputs["b_h2att"], dtype=np.float32).reshape(1, D)
    )
    wa = np.asarray(inputs["w_alpha"]).reshape(1, D).astype(bf)
    # b_alpha is a scalar added to every score; softmax is shift-invariant.
    onehots = np.ascontiguousarray(
        np.kron(np.eye(BPC, dtype=np.float32), np.ones((1, 128), dtype=np.float32))
    )
    in_maps = []
    for c in range(N_CORES):
        lo, hi = c * BPC, (c + 1) * BPC
        in_maps.append(
            {
                "onehots": onehots,
                "h": h[lo:hi],
                "att": att[lo:hi],
                "p": p[lo:hi],
                "w_h2att": W,
                "b_h2att": b2,
                "w_alpha": wa,
            }
        )
    return in_maps


def kernel(**inputs) -> np.ndarray:
    from concourse.bass_utils import run_bass_kernel_spmd

    nc = _get_program()
    in_maps = _make_in_maps(inputs)
    res = run_bass_kernel_spmd(nc, in_maps, list(range(N_CORES)))
    out = np.concatenate([res.results[c]["out"] for c in range(N_CORES)], axis=0)
    return out.astype(np.float32)


# revision 17
# speedup vs baseline: 1.2097x; 1.1775x over previous
"""ContentAttention kernel for 8 Trainium2 NeuronCores.

Computation (per batch b):
    h_att  = h[b] @ W_h2att + b_h2att                  # [512]
    e      = tanh(p_att_feats[b] + h_att)              # [1024, 512]
    scores = e @ w_alpha (+ b_alpha, dropped: softmax shift-invariant)
    w      = softmax(scores)                           # [1024]
    out[b] = w @ att_feats[b]                          # [1024]

Sharding: data-parallel over batch B=128 -> 16 batches/core x 8 cores.
Params are tiny and replicated.

The kernel is HBM-DMA-bound, so the two big streams (p_att_feats,
att_feats) and W are cast to bf16 on the host before upload: ~49MB of
HBM reads per core instead of 98MB (output rel-err ~2.8e-3, well under
the 2e-2 gate; accumulations stay fp32 in PSUM / DVE accum registers).

Per-core design:
  - regions are mapped partition-major (r = p*8 + j) so each batch's
    slab is one contiguous 8KB (p) / 16KB (att) run per partition ->
    one DMA descriptor per partition per slab. Softmax and the
    weighted sum are invariant to any consistent region permutation.
  - software-pipelined DMA issue: p slabs (1MB, ACT HWDGE ring) are
    issued 4 batches ahead, att slabs (2MB, SP HWDGE ring) 3 batches
    ahead, so slot-semaphore waits always reference long-consumed
    buffers and the loads never serialize behind same-engine compute.
  - h_att[b] broadcast across partitions via a onehot-selector matmul
    on PE into PSUM + ACT copy to bf16 SBUF.
  - DVE add -> ACT tanh -> per-chunk DVE tensor_tensor_reduce
    (fused *w_alpha multiply + free-dim sum) giving [128,1] score cols.
  - softmax over 1024 scores held as [128, 8], with no max-subtraction:
    |score| <= sum|w_alpha| (~18 since tanh is in [-1,1]), so exp
    cannot overflow fp32. exp + row-sum fused on ACT (accum_out);
    cross-partition sum via gpsimd partition_all_reduce; weights
    downcast to bf16.
  - weighted sum over regions on PE: lhsT = weight column [128,1] bf16,
    rhs = att slab chunk [128, 512] bf16, accumulated over 8 region
    chunks into PSUM [1, 512] x2.
"""

import numpy as np

B, R, K_H, D, F = 128, 1024, 1024, 512, 1024
N_CORES = 8
BPC = B // N_CORES  # batches per core
RC = R // 128  # region chunks per batch (r = p*RC + j)
KC = K_H // 128
PREF_P = 3  # p-slab DMA issue lead (batches); att is issued at its own
# iteration and consumed 4 stages later, so it needs no extra lead.

_cached = {}


def _build_program():
    from contextlib import ExitStack

    import concourse.bass as bass
    import concourse.bass_isa as bass_isa
    import concourse.tile as tile
    from concourse import bacc, mybir

    f32 = mybir.dt.float32
    bf16 = mybir.dt.bfloat16
    AF = mybir.ActivationFunctionType
    ALU = mybir.AluOpType
    AX = mybir.AxisListType

    def bcast_p(row_ap, parts):
        # replicate a [1, n] DRAM row across `parts` partitions
        return bass.AP(
            tensor=row_ap.tensor,
            offset=row_ap.offset,
            ap=[[0, parts], list(row_ap.ap[-1])],
        )

    def bcast_mid(t, c):
        # [128, D] sbuf tile -> [128, c, D] view with 0-stride middle dim
        return bass.AP(
            tensor=t.tensor,
            offset=t.offset,
            ap=[list(t.ap[0]), [0, c], list(t.ap[1])],
        )

    nc = bacc.Bacc("TRN2", target_bir_lowering=False, debug=False)
    oh_ap = nc.dram_tensor("onehots", [BPC, BPC * 128], f32, kind="ExternalInput").ap()
    h_ap = nc.dram_tensor("h", [BPC, K_H], f32, kind="ExternalInput").ap()
    att_ap = nc.dram_tensor("att", [BPC, R, F], bf16, kind="ExternalInput").ap()
    p_ap = nc.dram_tensor("p", [BPC, R, D], bf16, kind="ExternalInput").ap()
    w_ap = nc.dram_tensor("w_h2att", [K_H, D], bf16, kind="ExternalInput").ap()
    b2_ap = nc.dram_tensor("b_h2att", [1, D], f32, kind="ExternalInput").ap()
    wa_ap = nc.dram_tensor("w_alpha", [1, D], bf16, kind="ExternalInput").ap()
    out_ap = nc.dram_tensor("out", [BPC, F], f32, kind="ExternalOutput").ap()

    with tile.TileContext(nc) as tc, ExitStack() as ctx:
        consts = ctx.enter_context(tc.tile_pool(name="consts", bufs=1))
        wpool = ctx.enter_context(tc.tile_pool(name="wpool", bufs=1))
        ppool = ctx.enter_context(tc.tile_pool(name="ppool", bufs=6))
        apool = ctx.enter_context(tc.tile_pool(name="apool", bufs=6))
        hbpool = ctx.enter_context(tc.tile_pool(name="hbpool", bufs=3))
        spool = ctx.enter_context(tc.tile_pool(name="spool", bufs=6))
        outp = ctx.enter_context(tc.tile_pool(name="outp", bufs=3))
        ps_mm = ctx.enter_context(tc.tile_pool(name="ps_mm", bufs=2, space="PSUM"))
        ps_bc = ctx.enter_context(tc.tile_pool(name="ps_bc", bufs=2, space="PSUM"))
        ps_mic = ctx.enter_context(tc.tile_pool(name="ps_mic", bufs=1, space="PSUM"))

        # ---- prefetched big-stream DMAs, issued before any compute
        p_tiles, a_tiles = {}, {}

        def issue_p(b):
            t = ppool.tile([128, RC, D], bf16, tag="p")
            nc.scalar.dma_start(
                out=t, in_=p_ap[b].rearrange("(p j) d -> p j d", p=128)
            )
            p_tiles[b] = t

        def issue_a(b):
            t = apool.tile([128, RC, F], bf16, tag="a")
            nc.sync.dma_start(
                out=t, in_=att_ap[b].rearrange("(p j) f -> p j f", p=128)
            )
            a_tiles[b] = t

        issue_p(0)
        h_nat = wpool.tile([BPC, K_H], f32, tag="hnat")
        nc.scalar.dma_start(out=h_nat, in_=h_ap)
        w_all = wpool.tile([128, KC, D], bf16, tag="w")
        nc.sync.dma_start(
            out=w_all, in_=w_ap.rearrange("(kc p) d -> p kc d", p=128)
        )
        for b in range(1, PREF_P):
            issue_p(b)

        walpha_bc = consts.tile([128, D], bf16)
        nc.gpsimd.dma_start(out=walpha_bc, in_=bcast_p(wa_ap, 128))
        b2_bc = consts.tile([BPC, D], f32)
        nc.gpsimd.dma_start(out=b2_bc, in_=bcast_p(b2_ap, BPC))
        # onehots[:, b*128:(b+1)*128] is delta_{k,b} as a [16, 128] lhsT:
        # matmul against hatt broadcasts row b across all 128 partitions.
        onehots = consts.tile([BPC, BPC * 128], f32)
        nc.gpsimd.dma_start(out=onehots, in_=oh_ap)

        # ---- phase 0: h_att = h @ W + b_h2att, kept in SBUF
        ident16 = consts.tile([BPC, BPC], f32)
        from concourse.masks import make_identity

        make_identity(nc, ident16)
        # transpose h on PE: [16, 128] chunks -> [128, 16], downcast to bf16
        hT_all = wpool.tile([128, KC, BPC], bf16, tag="hT")
        for kc in range(KC):
            tr_ps = ps_mm.tile([128, BPC], f32, tag="ps0")
            nc.tensor.transpose(
                tr_ps, h_nat[:, kc * 128 : (kc + 1) * 128], ident16
            )
            nc.scalar.copy(hT_all[:, kc, :], tr_ps)
        ps_hatt = ps_mic.tile([BPC, D], f32, tag="mic")
        for kc in range(KC):
            nc.tensor.matmul(
                ps_hatt,
                lhsT=hT_all[:, kc, :],
                rhs=w_all[:, kc, :],
                start=(kc == 0),
                stop=(kc == KC - 1),
            )
        hatt = consts.tile([BPC, D], f32)
        nc.vector.tensor_add(hatt, ps_hatt, b2_bc)

        # ---- main loop: 6-stage skewed software pipeline.
        # Each stage runs one iteration after the stage feeding it, so
        # every engine's in-order instruction stream only reaches ops
        # whose cross-engine inputs were produced in a PREVIOUS
        # iteration (or earlier in this one, for the intra-iteration
        # p->add->tanh chain). This removes the stalls where e.g. DVE
        # sat blocked on the gpsimd all-reduce before it could start
        # the next batch's add.
        #   T1(b):   bc matmul [PE], hb copy [ACT], add [DVE], tanh [ACT]
        #   T2(b-1): e *= w_alpha [DVE], scores = row-sums [DVE]
        #   T3(b-2): exp + row-sum [ACT], cross-partition sum [gpsimd]
        #   T4(b-3): reciprocal, weights -> bf16 [DVE]
        #   T5(b-4): weighted-sum matmuls [PE]
        #   T6(b-5): PSUM -> SBUF copies [ACT], out store [gpsimd]
        scores_t, expb_t, sm_t, wgt_t, ps_t, ob_t = {}, {}, {}, {}, {}, {}
        for i in range(BPC + 5):
            if i < BPC:
                b = i
                if b + PREF_P < BPC:
                    issue_p(b + PREF_P)
                issue_a(b)

                # T1: broadcast h_att[b] on PE, downcast, add, tanh
                bc_ps = ps_bc.tile([128, D], f32, tag="bc")
                nc.tensor.matmul(
                    bc_ps,
                    lhsT=onehots[:, b * 128 : (b + 1) * 128],
                    rhs=hatt,
                    start=True,
                    stop=True,
                )
                hb = hbpool.tile([128, D], bf16)
                nc.scalar.copy(hb, bc_ps)
                p_t = p_tiles[b]
                nc.vector.tensor_add(p_t, p_t, bcast_mid(hb, RC))
                nc.scalar.activation(p_t, p_t, AF.Tanh)

            if 0 <= i - 1 < BPC:
                b = i - 1
                p_t = p_tiles.pop(b)
                nc.vector.tensor_mul(p_t, p_t, bcast_mid(walpha_bc, RC))
                scores = spool.tile([128, RC], f32, tag="scores")
                nc.vector.tensor_reduce(
                    scores, p_t, axis=AX.X, op=ALU.add
                )
                scores_t[b] = scores

            if 0 <= i - 2 < BPC:
                b = i - 2
                # softmax without max-subtraction (|score| <= ~18)
                expb = spool.tile([128, RC], f32, tag="expb")
                s1 = spool.tile([128, 1], f32, tag="s1")
                nc.scalar.activation(
                    expb, scores_t.pop(b), AF.Exp, accum_out=s1
                )
                sm = spool.tile([128, 1], f32, tag="sm")
                nc.gpsimd.partition_all_reduce(
                    sm, s1, channels=128, reduce_op=bass_isa.ReduceOp.add
                )
                expb_t[b], sm_t[b] = expb, sm

            if 0 <= i - 3 < BPC:
                b = i - 3
                rec = spool.tile([128, 1], f32, tag="rec")
                nc.vector.reciprocal(rec, sm_t.pop(b))
                wgt = spool.tile([128, RC], bf16, tag="wgt")
                nc.vector.tensor_scalar_mul(wgt, expb_t.pop(b), rec)
                wgt_t[b] = wgt

            if 0 <= i - 4 < BPC:
                b = i - 4
                a_t = a_tiles.pop(b)
                wgt = wgt_t.pop(b)
                ps0 = ps_mm.tile([1, 512], f32, tag="ps0")
                ps1 = ps_mm.tile([1, 512], f32, tag="ps1")
                for j in range(RC):
                    nc.tensor.matmul(
                        ps0,
                        lhsT=wgt[:, j : j + 1],
                        rhs=a_t[:, j, 0:512],
                        start=(j == 0),
                        stop=(j == RC - 1),
                    )
                    nc.tensor.matmul(
                        ps1,
                        lhsT=wgt[:, j : j + 1],
                        rhs=a_t[:, j, 512:1024],
                        start=(j == 0),
                        stop=(j == RC - 1),
                    )
                ps_t[b] = (ps0, ps1)

            if 0 <= i - 5 < BPC:
                b = i - 5
                ps0, ps1 = ps_t.pop(b)
                ob = outp.tile([1, F], f32)
                nc.scalar.copy(ob[:, 0:512], ps0)
                nc.scalar.copy(ob[:, 512:1024], ps1)
                nc.gpsimd.dma_start(out=out_ap[b : b + 1, :], in_=ob)

    nc.compile()
    return nc


def _get_program():
    if "nc" not in _cached:
        _cached["nc"] = _build_program()
    return _cached["nc"]


def _make_in_maps(inputs):
    import ml_dtypes

    bf = ml_dtypes.bfloat16
    h = np.ascontiguousarray(np.asarray(inputs["h"], dtype=np.float32))
    att = np.ascontiguousarray(np.asarray(inputs["att_feats"])).astype(bf)
    p = np.ascontiguousarray(np.asarray(inputs["p_att_feats"])).astype(bf)
    W = np.ascontiguousarray(np.asarray(inputs["W_h2att"])).astype(bf)
    b2 = np.ascontiguousarray(
        np.asarray(inputs["b_h2att"], dtype=np.float32).reshape(1, D)
    )
    wa = np.asarray(inputs["w_alpha"]).reshape(1, D).astype(bf)
    # b_alpha is a scalar added to every score; softmax is shift-invariant.
    onehots = np.ascontiguousarray(
        np.kron(np.eye(BPC, dtype=np.float32), np.ones((1, 128), dtype=np.float32))
    )
    in_maps = []
    for c in range(N_CORES):
        lo, hi = c * BPC, (c + 1) * BPC
        in_maps.append(
            {
                "onehots": onehots,
                "h": h[lo:hi],
                "att": att[lo:hi],
                "p": p[lo:hi],
                "w_h2att": W,
                "b_h2att": b2,
                "w_alpha": wa,
            }
        )
    return in_maps


def kernel(**inputs) -> np.ndarray:
    from concourse.bass_utils import run_bass_kernel_spmd

    nc = _get_program()
    in_maps = _make_in_maps(inputs)
    res = run_bass_kernel_spmd(nc, in_maps, list(range(N_CORES)))
    out = np.concatenate([res.results[c]["out"] for c in range(N_CORES)], axis=0)
    return out.astype(np.float32)


# revision 22
# speedup vs baseline: 1.2109x; 1.0010x over previous
"""ContentAttention kernel for 8 Trainium2 NeuronCores.

Computation (per batch b):
    h_att  = h[b] @ W_h2att + b_h2att                  # [512]
    e      = tanh(p_att_feats[b] + h_att)              # [1024, 512]
    scores = e @ w_alpha (+ b_alpha, dropped: softmax shift-invariant)
    w      = softmax(scores)                           # [1024]
    out[b] = w @ att_feats[b]                          # [1024]

Sharding: data-parallel over batch B=128 -> 16 batches/core x 8 cores.
Params are tiny and replicated.

The kernel is HBM-DMA-bound, so the two big streams (p_att_feats,
att_feats) and W are cast to bf16 on the host before upload: ~49MB of
HBM reads per core instead of 98MB (output rel-err ~2.8e-3, well under
the 2e-2 gate; accumulations stay fp32 in PSUM / DVE accum registers).

Per-core design:
  - regions are mapped partition-major (r = p*8 + j) so each batch's
    slab is one contiguous 8KB (p) / 16KB (att) run per partition ->
    one DMA descriptor per partition per slab. Softmax and the
    weighted sum are invariant to any consistent region permutation.
  - software-pipelined DMA issue: p slabs (1MB, ACT HWDGE ring) are
    issued 4 batches ahead, att slabs (2MB, SP HWDGE ring) 3 batches
    ahead, so slot-semaphore waits always reference long-consumed
    buffers and the loads never serialize behind same-engine compute.
  - h_att[b] broadcast across partitions via a onehot-selector matmul
    on PE into PSUM + ACT copy to bf16 SBUF.
  - DVE add -> ACT tanh -> per-chunk DVE tensor_tensor_reduce
    (fused *w_alpha multiply + free-dim sum) giving [128,1] score cols.
  - softmax over 1024 scores held as [128, 8], with no max-subtraction:
    |score| <= sum|w_alpha| (~18 since tanh is in [-1,1]), so exp
    cannot overflow fp32. exp + row-sum fused on ACT (accum_out);
    cross-partition sum via gpsimd partition_all_reduce; weights
    downcast to bf16.
  - weighted sum over regions on PE: lhsT = weight column [128,1] bf16,
    rhs = att slab chunk [128, 512] bf16, accumulated over 8 region
    chunks into PSUM [1, 512] x2.
"""

import numpy as np

B, R, K_H, D, F = 128, 1024, 1024, 512, 1024
N_CORES = 8
BPC = B // N_CORES  # batches per core
RC = R // 128  # region chunks per batch (r = p*RC + j)
KC = K_H // 128
PREF_P = 3  # p-slab DMA issue lead (batches)
PREF_A = 2  # att-slab DMA issue lead; consumption is 4 stages behind,
# so the effective lead over the consuming matmuls is PREF_A + 4.

_cached = {}


def _build_program():
    from contextlib import ExitStack

    import concourse.bass as bass
    import concourse.bass_isa as bass_isa
    import concourse.tile as tile
    from concourse import bacc, mybir

    f32 = mybir.dt.float32
    bf16 = mybir.dt.bfloat16
    AF = mybir.ActivationFunctionType
    ALU = mybir.AluOpType
    AX = mybir.AxisListType

    def bcast_p(row_ap, parts):
        # replicate a [1, n] DRAM row across `parts` partitions
        return bass.AP(
            tensor=row_ap.tensor,
            offset=row_ap.offset,
            ap=[[0, parts], list(row_ap.ap[-1])],
        )

    def bcast_mid(t, c):
        # [128, D] sbuf tile -> [128, c, D] view with 0-stride middle dim
        return bass.AP(
            tensor=t.tensor,
            offset=t.offset,
            ap=[list(t.ap[0]), [0, c], list(t.ap[1])],
        )

    nc = bacc.Bacc("TRN2", target_bir_lowering=False, debug=False)
    oh_ap = nc.dram_tensor("onehots", [BPC, BPC * 128], f32, kind="ExternalInput").ap()
    h_ap = nc.dram_tensor("h", [BPC, K_H], f32, kind="ExternalInput").ap()
    att_ap = nc.dram_tensor("att", [BPC, R, F], bf16, kind="ExternalInput").ap()
    p_ap = nc.dram_tensor("p", [BPC, R, D], bf16, kind="ExternalInput").ap()
    w_ap = nc.dram_tensor("w_h2att", [K_H, D], bf16, kind="ExternalInput").ap()
    b2_ap = nc.dram_tensor("b_h2att", [1, D], f32, kind="ExternalInput").ap()
    wa_ap = nc.dram_tensor("w_alpha", [1, D], bf16, kind="ExternalInput").ap()
    out_ap = nc.dram_tensor("out", [BPC, F], f32, kind="ExternalOutput").ap()

    with tile.TileContext(nc) as tc, ExitStack() as ctx:
        consts = ctx.enter_context(tc.tile_pool(name="consts", bufs=1))
        wpool = ctx.enter_context(tc.tile_pool(name="wpool", bufs=1))
        ppool = ctx.enter_context(tc.tile_pool(name="ppool", bufs=6))
        apool = ctx.enter_context(tc.tile_pool(name="apool", bufs=7))
        hbpool = ctx.enter_context(tc.tile_pool(name="hbpool", bufs=3))
        spool = ctx.enter_context(tc.tile_pool(name="spool", bufs=6))
        outp = ctx.enter_context(tc.tile_pool(name="outp", bufs=2))
        ps_mm = ctx.enter_context(tc.tile_pool(name="ps_mm", bufs=2, space="PSUM"))
        ps_bc = ctx.enter_context(tc.tile_pool(name="ps_bc", bufs=2, space="PSUM"))
        ps_mic = ctx.enter_context(tc.tile_pool(name="ps_mic", bufs=1, space="PSUM"))

        # ---- prefetched big-stream DMAs, issued before any compute
        p_tiles, a_tiles = {}, {}

        def issue_p(b):
            t = ppool.tile([128, RC, D], bf16, tag="p")
            nc.scalar.dma_start(
                out=t, in_=p_ap[b].rearrange("(p j) d -> p j d", p=128)
            )
            p_tiles[b] = t

        def issue_a(b):
            t = apool.tile([128, RC, F], bf16, tag="a")
            nc.sync.dma_start(
                out=t, in_=att_ap[b].rearrange("(p j) f -> p j f", p=128)
            )
            a_tiles[b] = t

        issue_p(0)
        h_nat = wpool.tile([BPC, K_H], f32, tag="hnat")
        nc.scalar.dma_start(out=h_nat, in_=h_ap)
        w_all = wpool.tile([128, KC, D], bf16, tag="w")
        nc.sync.dma_start(
            out=w_all, in_=w_ap.rearrange("(kc p) d -> p kc d", p=128)
        )
        for b in range(1, PREF_P):
            issue_p(b)
        for b in range(PREF_A):
            issue_a(b)

        walpha_bc = consts.tile([128, D], bf16)
        nc.gpsimd.dma_start(out=walpha_bc, in_=bcast_p(wa_ap, 128))
        b2_bc = consts.tile([BPC, D], f32)
        nc.gpsimd.dma_start(out=b2_bc, in_=bcast_p(b2_ap, BPC))
        # onehots[:, b*128:(b+1)*128] is delta_{k,b} as a [16, 128] lhsT:
        # matmul against hatt broadcasts row b across all 128 partitions.
        onehots = consts.tile([BPC, BPC * 128], f32)
        nc.gpsimd.dma_start(out=onehots, in_=oh_ap)

        # ---- phase 0: h_att = h @ W + b_h2att, kept in SBUF
        ident16 = consts.tile([BPC, BPC], f32)
        from concourse.masks import make_identity

        make_identity(nc, ident16)
        # transpose h on PE: [16, 128] chunks -> [128, 16], downcast to bf16
        hT_all = wpool.tile([128, KC, BPC], bf16, tag="hT")
        for kc in range(KC):
            tr_ps = ps_mm.tile([128, BPC], f32, tag="ps0")
            nc.tensor.transpose(
                tr_ps, h_nat[:, kc * 128 : (kc + 1) * 128], ident16
            )
            nc.scalar.copy(hT_all[:, kc, :], tr_ps)
        ps_hatt = ps_mic.tile([BPC, D], f32, tag="mic")
        for kc in range(KC):
            nc.tensor.matmul(
                ps_hatt,
                lhsT=hT_all[:, kc, :],
                rhs=w_all[:, kc, :],
                start=(kc == 0),
                stop=(kc == KC - 1),
            )
        hatt = consts.tile([BPC, D], f32)
        nc.vector.tensor_add(hatt, ps_hatt, b2_bc)

        # ---- main loop: 6-stage skewed software pipeline.
        # Each stage runs one iteration after the stage feeding it, so
        # every engine's in-order instruction stream only reaches ops
        # whose cross-engine inputs were produced in a PREVIOUS
        # iteration (or earlier in this one, for the intra-iteration
        # p->add->tanh chain). This removes the stalls where e.g. DVE
        # sat blocked on the gpsimd all-reduce before it could start
        # the next batch's add.
        #   T1(b):   bc matmul [PE], hb copy [ACT], add [DVE], tanh [ACT]
        #   T2(b-1): e *= w_alpha [DVE], scores = row-sums [DVE]
        #   T3(b-2): exp + row-sum [ACT], cross-partition sum [gpsimd]
        #   T4(b-3): reciprocal, weights -> bf16 [DVE]
        #   T5(b-4): weighted-sum matmuls [PE]
        #   T6(b-5): PSUM -> SBUF copies [ACT], out store [gpsimd]
        scores_t, expb_t, sm_t, wgt_t, ps_t, ob_t = {}, {}, {}, {}, {}, {}
        for i in range(BPC + 5):
            if i < BPC:
                b = i
                if b + PREF_P < BPC:
                    issue_p(b + PREF_P)
                if b + PREF_A < BPC:
                    issue_a(b + PREF_A)

                # T1: broadcast h_att[b] on PE, downcast, add, tanh
                bc_ps = ps_bc.tile([128, D], f32, tag="bc")
                nc.tensor.matmul(
                    bc_ps,
                    lhsT=onehots[:, b * 128 : (b + 1) * 128],
                    rhs=hatt,
                    start=True,
                    stop=True,
                )
                hb = hbpool.tile([128, D], bf16)
                nc.scalar.copy(hb, bc_ps)
                p_t = p_tiles[b]
                nc.vector.tensor_add(p_t, p_t, bcast_mid(hb, RC))
                nc.scalar.activation(p_t, p_t, AF.Tanh)

            if 0 <= i - 1 < BPC:
                b = i - 1
                p_t = p_tiles.pop(b)
                nc.vector.tensor_mul(p_t, p_t, bcast_mid(walpha_bc, RC))
                # bf16 scores keep the reduce on the DVE 2-byte fast path;
                # each is a 512-term sum of ~1e-2-magnitude products, so
                # bf16 rounding (~0.4%) is far inside the 2e-2 gate.
                scores = spool.tile([128, RC], bf16, tag="scores")
                with nc.allow_low_precision("bf16 scores, 2e-2 rel-err gate"):
                    nc.vector.tensor_reduce(
                        scores, p_t, axis=AX.X, op=ALU.add
                    )
                scores_t[b] = scores

            if 0 <= i - 2 < BPC:
                b = i - 2
                # softmax without max-subtraction (|score| <= ~18)
                expb = spool.tile([128, RC], f32, tag="expb")
                s1 = spool.tile([128, 1], f32, tag="s1")
                nc.scalar.activation(
                    expb, scores_t.pop(b), AF.Exp, accum_out=s1
                )
                sm = spool.tile([128, 1], f32, tag="sm")
                nc.gpsimd.partition_all_reduce(
                    sm, s1, channels=128, reduce_op=bass_isa.ReduceOp.add
                )
                expb_t[b], sm_t[b] = expb, sm

            if 0 <= i - 3 < BPC:
                b = i - 3
                rec = spool.tile([128, 1], f32, tag="rec")
                nc.vector.reciprocal(rec, sm_t.pop(b))
                wgt = spool.tile([128, RC], bf16, tag="wgt")
                nc.vector.tensor_scalar_mul(wgt, expb_t.pop(b), rec)
                wgt_t[b] = wgt

            if 0 <= i - 4 < BPC:
                b = i - 4
                a_t = a_tiles.pop(b)
                wgt = wgt_t.pop(b)
                ps0 = ps_mm.tile([1, 512], f32, tag="ps0")
                ps1 = ps_mm.tile([1, 512], f32, tag="ps1")
                for j in range(RC):
                    nc.tensor.matmul(
                        ps0,
                        lhsT=wgt[:, j : j + 1],
                        rhs=a_t[:, j, 0:512],
                        start=(j == 0),
                        stop=(j == RC - 1),
                    )
                    nc.tensor.matmul(
                        ps1,
                        lhsT=wgt[:, j : j + 1],
                        rhs=a_t[:, j, 512:1024],
                        start=(j == 0),
                        stop=(j == RC - 1),
                    )
                ps_t[b] = (ps0, ps1)

            if 0 <= i - 5 < BPC:
                b = i - 5
                ps0, ps1 = ps_t.pop(b)
                ob = outp.tile([1, F], f32)
                nc.scalar.copy(ob[:, 0:512], ps0)
                nc.scalar.copy(ob[:, 512:1024], ps1)
                nc.gpsimd.dma_start(out=out_ap[b : b + 1, :], in_=ob)

    nc.compile()
    return nc


def _get_program():
    if "nc" not in _cached:
        _cached["nc"] = _build_program()
    return _cached["nc"]


def _make_in_maps(inputs):
    import ml_dtypes

    bf = ml_dtypes.bfloat16
    h = np.ascontiguousarray(np.asarray(inputs["h"], dtype=np.float32))
    att = np.ascontiguousarray(np.asarray(inputs["att_feats"])).astype(bf)
    p = np.ascontiguousarray(np.asarray(inputs["p_att_feats"])).astype(bf)
    W = np.ascontiguousarray(np.asarray(inputs["W_h2att"])).astype(bf)
    b2 = np.ascontiguousarray(
        np.asarray(inputs["b_h2att"], dtype=np.float32).reshape(1, D)
    )
    wa = np.asarray(inputs["w_alpha"]).reshape(1, D).astype(bf)
    # b_alpha is a scalar added to every score; softmax is shift-invariant.
    onehots = np.ascontiguousarray(
        np.kron(np.eye(BPC, dtype=np.float32), np.ones((1, 128), dtype=np.float32))
    )
    in_maps = []
    for c in range(N_CORES):
        lo, hi = c * BPC, (c + 1) * BPC
        in_maps.append(
            {
                "onehots": onehots,
                "h": h[lo:hi],
                "att": att[lo:hi],
                "p": p[lo:hi],
                "w_h2att": W,
                "b_h2att": b2,
                "w_alpha": wa,
            }
        )
    return in_maps


def kernel(**inputs) -> np.ndarray:
    from concourse.bass_utils import run_bass_kernel_spmd

    nc = _get_program()
    in_maps = _make_in_maps(inputs)
    res = run_bass_kernel_spmd(nc, in_maps, list(range(N_CORES)))
    out = np.concatenate([res.results[c]["out"] for c in range(N_CORES)], axis=0)
    return out.astype(np.float32)


# revision 26
# speedup vs baseline: 1.2541x; 1.0357x over previous
"""ContentAttention kernel for 8 Trainium2 NeuronCores.

Computation (per batch b):
    h_att  = h[b] @ W_h2att + b_h2att                  # [512]
    e      = tanh(p_att_feats[b] + h_att)              # [1024, 512]
    scores = e @ w_alpha (+ b_alpha, dropped: softmax shift-invariant)
    w      = softmax(scores)                           # [1024]
    out[b] = w @ att_feats[b]                          # [1024]

Sharding: data-parallel over batch B=128 -> 16 batches/core x 8 cores.
Params are tiny and replicated.

The kernel is HBM-DMA-bound, so the two big streams (p_att_feats,
att_feats) and W are cast to bf16 on the host before upload: ~49MB of
HBM reads per core instead of 98MB (output rel-err ~2.8e-3, well under
the 2e-2 gate; accumulations stay fp32 in PSUM / DVE accum registers).

Per-core design:
  - regions are mapped partition-major (r = p*8 + j) so each batch's
    slab is one contiguous 8KB (p) / 16KB (att) run per partition ->
    one DMA descriptor per partition per slab. Softmax and the
    weighted sum are invariant to any consistent region permutation.
  - software-pipelined DMA issue: p slabs (1MB, ACT HWDGE ring) are
    issued 4 batches ahead, att slabs (2MB, SP HWDGE ring) 3 batches
    ahead, so slot-semaphore waits always reference long-consumed
    buffers and the loads never serialize behind same-engine compute.
  - h_att[b] broadcast across partitions via a onehot-selector matmul
    on PE into PSUM + ACT copy to bf16 SBUF.
  - DVE add -> ACT tanh -> per-chunk DVE tensor_tensor_reduce
    (fused *w_alpha multiply + free-dim sum) giving [128,1] score cols.
  - softmax over 1024 scores held as [128, 8], with no max-subtraction:
    |score| <= sum|w_alpha| (~18 since tanh is in [-1,1]), so exp
    cannot overflow fp32. exp + row-sum fused on ACT (accum_out);
    cross-partition sum via gpsimd partition_all_reduce; weights
    downcast to bf16.
  - weighted sum over regions on PE: lhsT = weight column [128,1] bf16,
    rhs = att slab chunk [128, 512] bf16, accumulated over 8 region
    chunks into PSUM [1, 512] x2.
"""

import numpy as np

B, R, K_H, D, F = 128, 1024, 1024, 512, 1024
N_CORES = 8
BPC = B // N_CORES  # batches per core
RC = R // 128  # region chunks per batch (r = p*RC + j)
KC = K_H // 128
PREF_P = 2  # p-slab DMA issue lead (batches)
PREF_A = 3  # att-slab DMA issue lead; consumption is 4 stages behind,
# so att slabs finish loading well before the final p slabs and the
# drain tail is just the last batch's chain, not leftover att DMAs.

_cached = {}


def _build_program():
    from contextlib import ExitStack

    import concourse.bass as bass
    import concourse.bass_isa as bass_isa
    import concourse.tile as tile
    from concourse import bacc, mybir

    f32 = mybir.dt.float32
    bf16 = mybir.dt.bfloat16
    AF = mybir.ActivationFunctionType
    ALU = mybir.AluOpType
    AX = mybir.AxisListType

    def bcast_p(row_ap, parts):
        # replicate a [1, n] DRAM row across `parts` partitions
        return bass.AP(
            tensor=row_ap.tensor,
            offset=row_ap.offset,
            ap=[[0, parts], list(row_ap.ap[-1])],
        )

    def bcast_mid(t, c):
        # [128, D] sbuf tile -> [128, c, D] view with 0-stride middle dim
        return bass.AP(
            tensor=t.tensor,
            offset=t.offset,
            ap=[list(t.ap[0]), [0, c], list(t.ap[1])],
        )

    nc = bacc.Bacc("TRN2", target_bir_lowering=False, debug=False)
    oh_ap = nc.dram_tensor("onehots", [BPC, BPC * 128], bf16, kind="ExternalInput").ap()
    h_ap = nc.dram_tensor("h", [BPC, K_H], f32, kind="ExternalInput").ap()
    att_ap = nc.dram_tensor("att", [BPC, R, F], bf16, kind="ExternalInput").ap()
    p_ap = nc.dram_tensor("p", [BPC, R, D], bf16, kind="ExternalInput").ap()
    w_ap = nc.dram_tensor("w_h2att", [K_H, D], bf16, kind="ExternalInput").ap()
    b2_ap = nc.dram_tensor("b_h2att", [1, D], bf16, kind="ExternalInput").ap()
    wa_ap = nc.dram_tensor("w_alpha", [1, D], bf16, kind="ExternalInput").ap()
    out_ap = nc.dram_tensor("out", [BPC, F], f32, kind="ExternalOutput").ap()

    with tile.TileContext(nc) as tc, ExitStack() as ctx:
        consts = ctx.enter_context(tc.tile_pool(name="consts", bufs=1))
        wpool = ctx.enter_context(tc.tile_pool(name="wpool", bufs=1))
        ppool = ctx.enter_context(tc.tile_pool(name="ppool", bufs=5))
        apool = ctx.enter_context(tc.tile_pool(name="apool", bufs=8))
        hbpool = ctx.enter_context(tc.tile_pool(name="hbpool", bufs=3))
        spool = ctx.enter_context(tc.tile_pool(name="spool", bufs=6))
        outp = ctx.enter_context(tc.tile_pool(name="outp", bufs=2))
        ps_mm = ctx.enter_context(tc.tile_pool(name="ps_mm", bufs=2, space="PSUM"))
        ps_bc = ctx.enter_context(tc.tile_pool(name="ps_bc", bufs=2, space="PSUM"))
        ps_mic = ctx.enter_context(tc.tile_pool(name="ps_mic", bufs=1, space="PSUM"))

        # ---- prefetched big-stream DMAs, issued before any compute
        p_tiles, a_tiles = {}, {}

        def issue_p(b):
            t = ppool.tile([128, RC, D], bf16, tag="p")
            nc.scalar.dma_start(
                out=t, in_=p_ap[b].rearrange("(p j) d -> p j d", p=128)
            )
            p_tiles[b] = t

        def issue_a(b):
            t = apool.tile([128, RC, F], bf16, tag="a")
            nc.sync.dma_start(
                out=t, in_=att_ap[b].rearrange("(p j) f -> p j f", p=128)
            )
            a_tiles[b] = t

        issue_p(0)
        h_nat = wpool.tile([BPC, K_H], f32, tag="hnat")
        nc.scalar.dma_start(out=h_nat, in_=h_ap)
        w_all = wpool.tile([128, KC, D], bf16, tag="w")
        nc.sync.dma_start(
            out=w_all, in_=w_ap.rearrange("(kc p) d -> p kc d", p=128)
        )
        for b in range(1, PREF_P):
            issue_p(b)
        for b in range(PREF_A):
            issue_a(b)

        walpha_bc = consts.tile([128, D], bf16)
        nc.gpsimd.dma_start(out=walpha_bc, in_=bcast_p(wa_ap, 128))
        b2_bc = consts.tile([BPC, D], bf16)
        nc.gpsimd.dma_start(out=b2_bc, in_=bcast_p(b2_ap, BPC))
        # onehots[:, b*128:(b+1)*128] is delta_{k,b} as a [16, 128] lhsT:
        # matmul against hatt broadcasts row b across all 128 partitions.
        onehots = consts.tile([BPC, BPC * 128], bf16)
        nc.gpsimd.dma_start(out=onehots, in_=oh_ap)

        # ---- phase 0: h_att = h @ W + b_h2att, kept in SBUF
        ident16 = consts.tile([BPC, BPC], f32)
        from concourse.masks import make_identity

        make_identity(nc, ident16)
        # transpose h on PE: [16, 128] chunks -> [128, 16], downcast to bf16
        hT_all = wpool.tile([128, KC, BPC], bf16, tag="hT")
        for kc in range(KC):
            tr_ps = ps_mm.tile([128, BPC], f32, tag="ps0")
            nc.tensor.transpose(
                tr_ps, h_nat[:, kc * 128 : (kc + 1) * 128], ident16
            )
            nc.scalar.copy(hT_all[:, kc, :], tr_ps)
        ps_hatt = ps_mic.tile([BPC, D], f32, tag="mic")
        for kc in range(KC):
            nc.tensor.matmul(
                ps_hatt,
                lhsT=hT_all[:, kc, :],
                rhs=w_all[:, kc, :],
                start=(kc == 0),
                stop=(kc == KC - 1),
            )
        hatt = consts.tile([BPC, D], bf16)
        nc.vector.tensor_add(hatt, ps_hatt, b2_bc)

        # ---- main loop: 6-stage skewed software pipeline.
        # Each stage runs one iteration after the stage feeding it, so
        # every engine's in-order instruction stream only reaches ops
        # whose cross-engine inputs were produced in a PREVIOUS
        # iteration (or earlier in this one, for the intra-iteration
        # p->add->tanh chain). This removes the stalls where e.g. DVE
        # sat blocked on the gpsimd all-reduce before it could start
        # the next batch's add.
        #   T1(b):   bc matmul [PE], hb copy [ACT], add [DVE], tanh [ACT]
        #   T2(b-1): e *= w_alpha [DVE], scores = row-sums [DVE]
        #   T3(b-2): exp + row-sum [ACT], cross-partition sum [gpsimd]
        #   T4(b-3): reciprocal, weights -> bf16 [DVE]
        #   T5(b-4): weighted-sum matmuls [PE]
        #   T6(b-5): PSUM -> SBUF copies [ACT], out store [gpsimd]
        scores_t, expb_t, sm_t, wgt_t, ps_t, ob_t = {}, {}, {}, {}, {}, {}
        for i in range(BPC + 5):
            if i < BPC:
                b = i
                if b + PREF_P < BPC:
                    issue_p(b + PREF_P)
                if b + PREF_A < BPC:
                    issue_a(b + PREF_A)

                # T1: broadcast h_att[b] on PE, downcast, add, tanh
                bc_ps = ps_bc.tile([128, D], f32, tag="bc")
                nc.tensor.matmul(
                    bc_ps,
                    lhsT=onehots[:, b * 128 : (b + 1) * 128],
                    rhs=hatt,
                    start=True,
                    stop=True,
                )
                hb = hbpool.tile([128, D], bf16)
                nc.scalar.copy(hb, bc_ps)
                p_t = p_tiles[b]
                nc.vector.tensor_add(p_t, p_t, bcast_mid(hb, RC))
                nc.scalar.activation(p_t, p_t, AF.Tanh)

            if 0 <= i - 2 < BPC:
                b = i - 2
                # T3: softmax without max-subtraction (|score| <= ~18).
                # Emitted before T2 so ACT reaches ready ops first.
                expb = spool.tile([128, RC], f32, tag="expb")
                s1 = spool.tile([128, 1], f32, tag="s1")
                nc.scalar.activation(
                    expb, scores_t.pop(b), AF.Exp, accum_out=s1
                )
                sm = spool.tile([128, 1], f32, tag="sm")
                nc.gpsimd.partition_all_reduce(
                    sm, s1, channels=128, reduce_op=bass_isa.ReduceOp.add
                )
                expb_t[b], sm_t[b] = expb, sm

            if 0 <= i - 5 < BPC:
                b = i - 5
                # T6: drain PSUM accumulators, store out[b]
                ps0, ps1 = ps_t.pop(b)
                ob = outp.tile([1, F], f32)
                nc.scalar.copy(ob[:, 0:512], ps0)
                nc.scalar.copy(ob[:, 512:1024], ps1)
                nc.gpsimd.dma_start(out=out_ap[b : b + 1, :], in_=ob)

            if 0 <= i - 1 < BPC:
                b = i - 1
                # T2: e *= w_alpha, then per-chunk row-sums -> scores.
                # bf16 scores: each is a 512-term sum of small products,
                # bf16 rounding (~0.4%) is far inside the 2e-2 gate.
                # The reduce is split 6 chunks on DVE (the pipeline's
                # pacing engine) + 2 chunks on ACT via Copy+accum.
                p_t = p_tiles.pop(b)
                nc.vector.tensor_mul(p_t, p_t, bcast_mid(walpha_bc, RC))
                scores = spool.tile([128, RC], bf16, tag="scores")
                with nc.allow_low_precision("bf16 scores, 2e-2 rel-err gate"):
                    nc.vector.tensor_reduce(
                        scores[:, 0:6], p_t[:, 0:6, :], axis=AX.X, op=ALU.add
                    )
                    for j in range(6, RC):
                        nc.scalar.activation(
                            p_t[:, j, :],
                            p_t[:, j, :],
                            AF.Copy,
                            accum_out=scores[:, j : j + 1],
                        )
                scores_t[b] = scores

            if 0 <= i - 3 < BPC:
                b = i - 3
                # T4: normalization factors, weights in bf16
                rec = spool.tile([128, 1], f32, tag="rec")
                nc.vector.reciprocal(rec, sm_t.pop(b))
                wgt = spool.tile([128, RC], bf16, tag="wgt")
                nc.vector.tensor_scalar_mul(wgt, expb_t.pop(b), rec)
                wgt_t[b] = wgt

            if 0 <= i - 4 < BPC:
                b = i - 4
                a_t = a_tiles.pop(b)
                wgt = wgt_t.pop(b)
                ps0 = ps_mm.tile([1, 512], f32, tag="ps0")
                ps1 = ps_mm.tile([1, 512], f32, tag="ps1")
                for j in range(RC):
                    nc.tensor.matmul(
                        ps0,
                        lhsT=wgt[:, j : j + 1],
                        rhs=a_t[:, j, 0:512],
                        start=(j == 0),
                        stop=(j == RC - 1),
                    )
                    nc.tensor.matmul(
                        ps1,
                        lhsT=wgt[:, j : j + 1],
                        rhs=a_t[:, j, 512:1024],
                        start=(j == 0),
                        stop=(j == RC - 1),
                    )
                ps_t[b] = (ps0, ps1)

    nc.compile()
    return nc


def _get_program():
    if "nc" not in _cached:
        _cached["nc"] = _build_program()
    return _cached["nc"]


def _make_in_maps(inputs):
    import ml_dtypes

    bf = ml_dtypes.bfloat16
    h = np.ascontiguousarray(np.asarray(inputs["h"], dtype=np.float32))
    att = np.ascontiguousarray(np.asarray(inputs["att_feats"])).astype(bf)
    p = np.ascontiguousarray(np.asarray(inputs["p_att_feats"])).astype(bf)
    W = np.ascontiguousarray(np.asarray(inputs["W_h2att"])).astype(bf)
    b2 = np.asarray(inputs["b_h2att"]).reshape(1, D).astype(bf)
    wa = np.asarray(inputs["w_alpha"]).reshape(1, D).astype(bf)
    # b_alpha is a scalar added to every score; softmax is shift-invariant.
    onehots = np.ascontiguousarray(
        np.kron(np.eye(BPC, dtype=np.float32), np.ones((1, 128), dtype=np.float32))
    ).astype(bf)
    in_maps = []
    for c in range(N_CORES):
        lo, hi = c * BPC, (c + 1) * BPC
        in_maps.append(
            {
                "onehots": onehots,
                "h": h[lo:hi],
                "att": att[lo:hi],
                "p": p[lo:hi],
                "w_h2att": W,
                "b_h2att": b2,
                "w_alpha": wa,
            }
        )
    return in_maps


def kernel(**inputs) -> np.ndarray:
    from concourse.bass_utils import run_bass_kernel_spmd

    nc = _get_program()
    in_maps = _make_in_maps(inputs)
    res = run_bass_kernel_spmd(nc, in_maps, list(range(N_CORES)))
    out = np.concatenate([res.results[c]["out"] for c in range(N_CORES)], axis=0)
    return out.astype(np.float32)


# revision 28
# speedup vs baseline: 1.3485x; 1.0753x over previous
"""ContentAttention kernel for 8 Trainium2 NeuronCores.

Computation (per batch b):
    h_att  = h[b] @ W_h2att + b_h2att                  # [512]
    e      = tanh(p_att_feats[b] + h_att)              # [1024, 512]
    scores = e @ w_alpha (+ b_alpha, dropped: softmax shift-invariant)
    w      = softmax(scores)                           # [1024]
    out[b] = w @ att_feats[b]                          # [1024]

Sharding: data-parallel over batch B=128 -> 16 batches/core x 8 cores.
Params are tiny and replicated.

The kernel is HBM-DMA-bound, so the two big streams (p_att_feats,
att_feats) and W are cast to bf16 on the host before upload: ~49MB of
HBM reads per core instead of 98MB (output rel-err ~2.8e-3, well under
the 2e-2 gate; accumulations stay fp32 in PSUM / DVE accum registers).

Per-core design:
  - regions are mapped partition-major (r = p*8 + j) so each batch's
    slab is one contiguous 8KB (p) / 16KB (att) run per partition ->
    one DMA descriptor per partition per slab. Softmax and the
    weighted sum are invariant to any consistent region permutation.
  - software-pipelined DMA issue: p slabs (1MB, ACT HWDGE ring) are
    issued 4 batches ahead, att slabs (2MB, SP HWDGE ring) 3 batches
    ahead, so slot-semaphore waits always reference long-consumed
    buffers and the loads never serialize behind same-engine compute.
  - h_att[b] broadcast across partitions via a onehot-selector matmul
    on PE into PSUM + ACT copy to bf16 SBUF.
  - DVE add -> ACT tanh -> per-chunk DVE tensor_tensor_reduce
    (fused *w_alpha multiply + free-dim sum) giving [128,1] score cols.
  - softmax over 1024 scores held as [128, 8], with no max-subtraction:
    |score| <= sum|w_alpha| (~18 since tanh is in [-1,1]), so exp
    cannot overflow fp32. exp + row-sum fused on ACT (accum_out);
    cross-partition sum via gpsimd partition_all_reduce; weights
    downcast to bf16.
  - weighted sum over regions on PE: lhsT = weight column [128,1] bf16,
    rhs = att slab chunk [128, 512] bf16, accumulated over 8 region
    chunks into PSUM [1, 512] x2.
"""

import numpy as np

B, R, K_H, D, F = 128, 1024, 1024, 512, 1024
N_CORES = 8
BPC = B // N_CORES  # batches per core
RC = R // 128  # region chunks per batch (r = p*RC + j)
KC = K_H // 128
PREF_P = 2  # p-slab DMA issue lead (batches)
PREF_A = 3  # att-slab DMA issue lead; consumption is 4 stages behind,
# so att slabs finish loading well before the final p slabs and the
# drain tail is just the last batch's chain, not leftover att DMAs.

_cached = {}


def _build_program():
    from contextlib import ExitStack

    import concourse.bass as bass
    import concourse.bass_isa as bass_isa
    import concourse.tile as tile
    from concourse import bacc, mybir

    f32 = mybir.dt.float32
    bf16 = mybir.dt.bfloat16
    AF = mybir.ActivationFunctionType
    ALU = mybir.AluOpType
    AX = mybir.AxisListType

    def bcast_p(row_ap, parts):
        # replicate a [1, n] DRAM row across `parts` partitions
        return bass.AP(
            tensor=row_ap.tensor,
            offset=row_ap.offset,
            ap=[[0, parts], list(row_ap.ap[-1])],
        )

    def bcast_mid(t, c):
        # [128, D] sbuf tile -> [128, c, D] view with 0-stride middle dim
        return bass.AP(
            tensor=t.tensor,
            offset=t.offset,
            ap=[list(t.ap[0]), [0, c], list(t.ap[1])],
        )

    nc = bacc.Bacc("TRN2", target_bir_lowering=False, debug=False)
    oh_ap = nc.dram_tensor("onehots", [BPC, BPC * 128], bf16, kind="ExternalInput").ap()
    h_ap = nc.dram_tensor("h", [BPC, K_H], f32, kind="ExternalInput").ap()
    att_ap = nc.dram_tensor("att", [BPC, R, F], bf16, kind="ExternalInput").ap()
    p_ap = nc.dram_tensor("p", [BPC, R, D], bf16, kind="ExternalInput").ap()
    w_ap = nc.dram_tensor("w_h2att", [K_H, D], bf16, kind="ExternalInput").ap()
    b2_ap = nc.dram_tensor("b_h2att", [1, D], bf16, kind="ExternalInput").ap()
    wa_ap = nc.dram_tensor("w_alpha", [1, D], bf16, kind="ExternalInput").ap()
    out_ap = nc.dram_tensor("out", [BPC, F], f32, kind="ExternalOutput").ap()

    with tile.TileContext(nc) as tc, ExitStack() as ctx:
        consts = ctx.enter_context(tc.tile_pool(name="consts", bufs=1))
        wpool = ctx.enter_context(tc.tile_pool(name="wpool", bufs=1))
        ppool = ctx.enter_context(tc.tile_pool(name="ppool", bufs=5))
        apool = ctx.enter_context(tc.tile_pool(name="apool", bufs=8))
        hbpool = ctx.enter_context(tc.tile_pool(name="hbpool", bufs=3))
        spool = ctx.enter_context(tc.tile_pool(name="spool", bufs=6))
        outp = ctx.enter_context(tc.tile_pool(name="outp", bufs=2))
        ps_mm = ctx.enter_context(tc.tile_pool(name="ps_mm", bufs=2, space="PSUM"))
        ps_bc = ctx.enter_context(tc.tile_pool(name="ps_bc", bufs=2, space="PSUM"))
        ps_mic = ctx.enter_context(tc.tile_pool(name="ps_mic", bufs=1, space="PSUM"))

        # ---- prefetched big-stream DMAs, issued before any compute
        p_tiles, a_tiles = {}, {}

        def issue_p(b):
            t = ppool.tile([128, RC, D], bf16, tag="p")
            nc.scalar.dma_start(
                out=t, in_=p_ap[b].rearrange("(p j) d -> p j d", p=128)
            )
            p_tiles[b] = t

        def issue_a(b):
            t = apool.tile([128, RC, F], bf16, tag="a")
            nc.sync.dma_start(
                out=t, in_=att_ap[b].rearrange("(p j) f -> p j f", p=128)
            )
            a_tiles[b] = t

        issue_p(0)
        h_nat = wpool.tile([BPC, K_H], f32, tag="hnat")
        nc.scalar.dma_start(out=h_nat, in_=h_ap)
        w_all = wpool.tile([128, KC, D], bf16, tag="w")
        nc.sync.dma_start(
            out=w_all, in_=w_ap.rearrange("(kc p) d -> p kc d", p=128)
        )
        for b in range(1, PREF_P):
            issue_p(b)
        for b in range(PREF_A):
            issue_a(b)

        walpha_bc = consts.tile([128, D], bf16)
        nc.gpsimd.dma_start(out=walpha_bc, in_=bcast_p(wa_ap, 128))
        b2_bc = consts.tile([BPC, D], bf16)
        nc.gpsimd.dma_start(out=b2_bc, in_=bcast_p(b2_ap, BPC))
        # onehots[:, b*128:(b+1)*128] is delta_{k,b} as a [16, 128] lhsT:
        # matmul against hatt broadcasts row b across all 128 partitions.
        onehots = consts.tile([BPC, BPC * 128], bf16)
        nc.gpsimd.dma_start(out=onehots, in_=oh_ap)

        # ---- phase 0: h_att = h @ W + b_h2att, kept in SBUF
        ident16 = consts.tile([BPC, BPC], f32)
        from concourse.masks import make_identity

        make_identity(nc, ident16)
        # transpose h on PE: [16, 128] chunks -> [128, 16], downcast to bf16
        hT_all = wpool.tile([128, KC, BPC], bf16, tag="hT")
        for kc in range(KC):
            tr_ps = ps_mm.tile([128, BPC], f32, tag="ps0")
            nc.tensor.transpose(
                tr_ps, h_nat[:, kc * 128 : (kc + 1) * 128], ident16
            )
            nc.scalar.copy(hT_all[:, kc, :], tr_ps)
        ps_hatt = ps_mic.tile([BPC, D], f32, tag="mic")
        for kc in range(KC):
            nc.tensor.matmul(
                ps_hatt,
                lhsT=hT_all[:, kc, :],
                rhs=w_all[:, kc, :],
                start=(kc == 0),
                stop=(kc == KC - 1),
            )
        hatt = consts.tile([BPC, D], bf16)
        nc.vector.tensor_add(hatt, ps_hatt, b2_bc)

        # ---- main loop: 6-stage skewed software pipeline.
        # Each stage runs one iteration after the stage feeding it, so
        # every engine's in-order instruction stream only reaches ops
        # whose cross-engine inputs were produced in a PREVIOUS
        # iteration (or earlier in this one, for the intra-iteration
        # p->add->tanh chain). This removes the stalls where e.g. DVE
        # sat blocked on the gpsimd all-reduce before it could start
        # the next batch's add.
        #   T1(b):   bc matmul [PE], hb copy [ACT], add [DVE], tanh [ACT]
        #   T2(b-1): e *= w_alpha [DVE], scores = row-sums [DVE]
        #   T3(b-2): exp + row-sum [ACT], cross-partition sum [gpsimd]
        #   T4(b-3): reciprocal, weights -> bf16 [DVE]
        #   T5(b-4): weighted-sum matmuls [PE]
        #   T6(b-5): PSUM -> SBUF copies [ACT], out store [gpsimd]
        def make_hb(b):
            # broadcast h_att[b] to 128 partitions on PE, downcast to bf16
            bc_ps = ps_bc.tile([128, D], f32, tag="bc")
            nc.tensor.matmul(
                bc_ps,
                lhsT=onehots[:, b * 128 : (b + 1) * 128],
                rhs=hatt,
                start=True,
                stop=True,
            )
            hb = hbpool.tile([128, D], bf16)
            nc.scalar.copy(hb, bc_ps)
            hb_t[b] = hb

        scores_t, expb_t, sm_t, wgt_t, ps_t, hb_t = {}, {}, {}, {}, {}, {}
        make_hb(0)
        for i in range(BPC + 5):
            if i < BPC:
                b = i
                if b + PREF_P < BPC:
                    issue_p(b + PREF_P)
                if b + PREF_A < BPC:
                    issue_a(b + PREF_A)
                # T0: next iteration's h_att broadcast, off the critical
                # path so add(b) finds hb(b) ready at iteration start
                if b + 1 < BPC:
                    make_hb(b + 1)
                # T1 (DVE part): p += h_att[b]
                p_t = p_tiles[b]
                nc.vector.tensor_add(p_t, p_t, bcast_mid(hb_t.pop(b), RC))

            if 0 <= i - 2 < BPC:
                b = i - 2
                # T3: softmax without max-subtraction (|score| <= ~18).
                # Emitted before T2 so ACT reaches ready ops first.
                expb = spool.tile([128, RC], f32, tag="expb")
                s1 = spool.tile([128, 1], f32, tag="s1")
                nc.scalar.activation(
                    expb, scores_t.pop(b), AF.Exp, accum_out=s1
                )
                sm = spool.tile([128, 1], f32, tag="sm")
                nc.gpsimd.partition_all_reduce(
                    sm, s1, channels=128, reduce_op=bass_isa.ReduceOp.add
                )
                expb_t[b], sm_t[b] = expb, sm

            if 0 <= i - 5 < BPC:
                b = i - 5
                # T6: drain PSUM accumulators, store out[b]
                ps0, ps1 = ps_t.pop(b)
                ob = outp.tile([1, F], f32)
                nc.scalar.copy(ob[:, 0:512], ps0)
                nc.scalar.copy(ob[:, 512:1024], ps1)
                nc.gpsimd.dma_start(out=out_ap[b : b + 1, :], in_=ob)

            if i < BPC:
                # T1 (ACT part): tanh, emitted after exp/obs so ACT does
                # ready work while DVE finishes add(b)
                nc.scalar.activation(p_tiles[i], p_tiles[i], AF.Tanh)

            if 0 <= i - 1 < BPC:
                b = i - 1
                # T2: e *= w_alpha, then per-chunk row-sums -> scores.
                # bf16 scores: each is a 512-term sum of small products,
                # bf16 rounding (~0.4%) is far inside the 2e-2 gate.
                # The reduce is split 6 chunks on DVE (the pipeline's
                # pacing engine) + 2 chunks on ACT via Copy+accum.
                p_t = p_tiles.pop(b)
                nc.vector.tensor_mul(p_t, p_t, bcast_mid(walpha_bc, RC))
                scores = spool.tile([128, RC], bf16, tag="scores")
                with nc.allow_low_precision("bf16 scores, 2e-2 rel-err gate"):
                    nc.vector.tensor_reduce(
                        scores[:, 0:6], p_t[:, 0:6, :], axis=AX.X, op=ALU.add
                    )
                    for j in range(6, RC):
                        nc.scalar.activation(
                            p_t[:, j, :],
                            p_t[:, j, :],
                            AF.Copy,
                            accum_out=scores[:, j : j + 1],
                        )
                scores_t[b] = scores

            if 0 <= i - 3 < BPC:
                b = i - 3
                # T4: normalization factors, weights in bf16
                rec = spool.tile([128, 1], f32, tag="rec")
                nc.vector.reciprocal(rec, sm_t.pop(b))
                wgt = spool.tile([128, RC], bf16, tag="wgt")
                nc.vector.tensor_scalar_mul(wgt, expb_t.pop(b), rec)
                wgt_t[b] = wgt

            if 0 <= i - 4 < BPC:
                b = i - 4
                a_t = a_tiles.pop(b)
                wgt = wgt_t.pop(b)
                ps0 = ps_mm.tile([1, 512], f32, tag="ps0")
                ps1 = ps_mm.tile([1, 512], f32, tag="ps1")
                for j in range(RC):
                    nc.tensor.matmul(
                        ps0,
                        lhsT=wgt[:, j : j + 1],
                        rhs=a_t[:, j, 0:512],
                        start=(j == 0),
                        stop=(j == RC - 1),
                    )
                    nc.tensor.matmul(
                        ps1,
                        lhsT=wgt[:, j : j + 1],
                        rhs=a_t[:, j, 512:1024],
                        start=(j == 0),
                        stop=(j == RC - 1),
                    )
                ps_t[b] = (ps0, ps1)

    nc.compile()
    return nc


def _get_program():
    if "nc" not in _cached:
        _cached["nc"] = _build_program()
    return _cached["nc"]


def _make_in_maps(inputs):
    import ml_dtypes

    bf = ml_dtypes.bfloat16
    h = np.ascontiguousarray(np.asarray(inputs["h"], dtype=np.float32))
    att = np.ascontiguousarray(np.asarray(inputs["att_feats"])).astype(bf)
    p = np.ascontiguousarray(np.asarray(inputs["p_att_feats"])).astype(bf)
    W = np.ascontiguousarray(np.asarray(inputs["W_h2att"])).astype(bf)
    b2 = np.asarray(inputs["b_h2att"]).reshape(1, D).astype(bf)
    wa = np.asarray(inputs["w_alpha"]).reshape(1, D).astype(bf)
    # b_alpha is a scalar added to every score; softmax is shift-invariant.
    onehots = np.ascontiguousarray(
        np.kron(np.eye(BPC, dtype=np.float32), np.ones((1, 128), dtype=np.float32))
    ).astype(bf)
    in_maps = []
    for c in range(N_CORES):
        lo, hi = c * BPC, (c + 1) * BPC
        in_maps.append(
            {
                "onehots": onehots,
                "h": h[lo:hi],
                "att": att[lo:hi],
                "p": p[lo:hi],
                "w_h2att": W,
                "b_h2att": b2,
                "w_alpha": wa,
            }
        )
    return in_maps


def kernel(**inputs) -> np.ndarray:
    from concourse.bass_utils import run_bass_kernel_spmd

    nc = _get_program()
    in_maps = _make_in_maps(inputs)
    res = run_bass_kernel_spmd(nc, in_maps, list(range(N_CORES)))
    out = np.concatenate([res.results[c]["out"] for c in range(N_CORES)], axis=0)
    return out.astype(np.float32)
